# revision 23
# baseline (speedup 1.0000x reference)
"""Conv2D-KAN Trainium2 kernel (8-core data-parallel SPMD).

Formulation
-----------
The reference computes, per 3x3 patch (N = B*30*30 patches, in_size = 288):
    out[n,o] = sum_{i,k} sb[n,i,k] * (spline_kernel*scale)[i,k,o]
             + silu(xf) @ scale_factor + biases
where sb is a cubic B-spline basis (8 funcs) over a uniform grid
(knots t_r = -2.2 + 0.4 r, r = 0..11, h = 0.4).

Key identities:
 1. Basis values depend only on the underlying *pixel*, not the patch
    (patch extraction is a gather), so features are computed per pixel
    (8x less elementwise work than per-patch).
 2. Uniform cubic B-splines decompose over truncated powers:
        B_k(x) = (1/6) sum_{m=0..4} cm_m T_{k+m}(x), cm = [1,-4,6,-4,1]
        T_r(x) = min(relu((x - t_r)/h), 11-r)^3
    The clamp at 11-r makes every B_k *exactly* zero outside the grid
    (integer cancellation), matching the reference's out-of-range
    behaviour without masks, and T_11 == 0 so only r = 0..10 exist.
 3. The whole op is then a 3x3 convolution with 128 filters over
    pixel-feature channels, done as accumulating 128-K matmuls into
    PSUM banks of [128 filters, 450 patches].

Two modes:
 * "fp32"  — features are the 11 truncated cubes + silu per channel
             (384 = 3x128 K-chunks per offset, 27 matmuls per bank),
             blending folded into the weights. Full fp32 matmuls
             (4 cyc/row). Max rel err ~1e-5.
 * "basis" — the blending T -> B_k happens on DVE in fp32 (exact), so
             the matmul operands are the well-conditioned basis values
             (<= 4) and the matmuls run in float32r (TF32-like, 1-pass,
             ~1.4 cyc/row). 8 basis + silu -> 2x128 + 32 K-chunks per
             offset, 27 matmuls per bank. Rel err ~ a few 1e-5.

Each core processes 4 images; output [128, 3600] per core is
transposed on host.
"""

import sys

sys.path.insert(0, "/opt/trn_rl_repo")

import numpy as np

N_CORES = 8
B, HH, WW, C = 32, 32, 32, 32
F = 128
KH = KW = 3
HO, WO = HH - KH + 1, WW - KW + 1          # 30, 30
BPC = B // N_CORES                          # images per core = 4
PIX = HH * WW                               # 1024 pixels per image
NPC = BPC * HO * WO                         # 3600 patches per core
NBANK = 2 * BPC                             # 8 psum banks
BANKN = NPC // NBANK                        # 450
HGRID = 0.4
T0 = -2.2                                   # first knot
NR = 11                                     # truncated-cube features
NFEAT = 12                                  # + silu
NMM = 27                                    # matmuls per bank (both modes)

MODE = "fp32"  # "fp32" | "basis"

_cache = {}


def _build_program(mode):
    import concourse.bacc as bacc
    import concourse.mybir as mybir
    import concourse.tile as tile

    f32 = mybir.dt.float32
    f32r = mybir.dt.float32r
    AF = mybir.ActivationFunctionType
    basis = mode == "basis"

    nch = NMM + 2 if basis else NMM
    nc = bacc.Bacc("TRN2", target_bir_lowering=False, debug=False)
    xt = nc.dram_tensor("xt", [C, BPC * PIX], f32, kind="ExternalInput").ap()
    # weights: [128 partitions, nch * F] -> one contiguous DMA
    wt = nc.dram_tensor("wt", [128, nch * F], f32, kind="ExternalInput").ap()
    consts = nc.dram_tensor("consts", [128, 8], f32, kind="ExternalInput").ap()
    y = nc.dram_tensor("y", [F, NPC], f32, kind="ExternalOutput").ap()

    with tile.TileContext(nc) as tc:
        with (
            tc.tile_pool(name="wp", bufs=1) as wp,
            tc.tile_pool(name="cp", bufs=1) as cp,
            tc.tile_pool(name="fp", bufs=3) as fp,
            tc.tile_pool(name="sp", bufs=3) as sp,
            tc.tile_pool(name="op", bufs=1) as op_,
            tc.tile_pool(name="pp", bufs=4, space="PSUM") as pp,
        ):
            ct = cp.tile([128, 8], f32)
            nc.gpsimd.dma_start(ct[:], consts[:])

            # weights on a separate DMA queue so the first feature tiles
            # (sync queue) are not stuck behind the 1.9MB weight load;
            # chunk 0 arrives first in its own small transfer.
            wbig = wp.tile([128, nch * F], f32, tag="wbig")
            nc.gpsimd.dma_start(wbig[:, :F], wt[:, :F])
            nc.gpsimd.dma_start(wbig[:, F:], wt[:, F:])
            if basis:
                wrbig = wp.tile([128, NMM * F], f32r, tag="wrbig")
                nc.vector.tensor_copy(wrbig[:], wbig[:, :NMM * F])
                wtiles = [wrbig[:, i * F:(i + 1) * F] for i in range(NMM)]
                Ma = wbig[:, NMM * F:(NMM + 1) * F]
                Mb = wbig[:, (NMM + 1) * F:(NMM + 2) * F]
            else:
                wtiles = [wbig[:, i * F:(i + 1) * F] for i in range(NMM)]

            out_t = op_.tile([F, NPC], f32)

            def banks(im, mk_rhs):
                for half in range(2):
                    ps = pp.tile([F, BANKN], f32, tag="ps")
                    k = 0
                    for off in range(KH * KW):
                        di, dj = divmod(off, KW)
                        h0 = half * 15 + di
                        for t in range(3):
                            lhsT, rhs = mk_rhs(off, t, h0, dj)
                            nc.tensor.matmul(
                                ps[:], lhsT, rhs,
                                start=(k == 0), stop=(k == NMM - 1),
                            )
                            k += 1
                    s = (im * 2 + half) * BANKN
                    nc.scalar.activation(
                        out_t[:, s:s + BANKN], ps[:], AF.Identity,
                        bias=ct[:, 6:7], scale=1.0,
                    )
                    nc.sync.dma_start(y[:, s:s + BANKN], out_t[:, s:s + BANKN])

            for im in range(BPC):
                sl = slice(im * PIX, (im + 1) * PIX)
                if basis:
                    # --- T tiles (same r-major 4r x 32c layout as fp32 mode)
                    Ts = []
                    for t in range(3):
                        T = fp.tile([128, PIX], f32, tag=f"T{t}")
                        src = (xt[:, sl].unsqueeze(0)
                               .broadcast_to((4, 32, PIX)))
                        nc.sync.dma_start(T[:], src)
                        nc.scalar.activation(
                            T[:], T[:], AF.Relu,
                            bias=ct[:, t:t + 1], scale=1.0 / HGRID)
                        nc.vector.tensor_scalar_min(
                            T[:], T[:], ct[:, 3 + t:4 + t])
                        sq = sp.tile([128, PIX], f32, tag="sq")
                        nc.scalar.activation(sq[:], T[:], AF.Square)
                        nc.vector.tensor_mul(T[:], sq[:], T[:])
                        Ts.append(T)
                    # --- combine B_k = sum_m cm_m T_{k+m} on PE:
                    # two banded constant matrices contract the r dim
                    # (engines cannot read shifted partition windows).
                    Bviews = []
                    for g in range(2):
                        Bt = fp.tile([128, PIX], f32r, tag=f"B{g}")
                        for hf in range(2):
                            hs = slice(hf * 512, (hf + 1) * 512)
                            bp = pp.tile([128, 512], f32, tag="psB")
                            nc.tensor.matmul(bp[:], Ma, Ts[g][:, hs],
                                             start=True, stop=False)
                            nc.tensor.matmul(bp[:], Mb, Ts[g + 1][:, hs],
                                             start=False, stop=True)
                            nc.scalar.activation(Bt[:, hs], bp[:], AF.Copy)
                        Bviews.append(
                            Bt[:].rearrange("p (h w) -> p h w", w=WW))
                    # --- silu ---
                    xs = sp.tile([32, PIX], f32, tag="xs")
                    nc.sync.dma_start(xs[:], xt[:, sl])
                    SL = fp.tile([32, PIX], f32r, tag="SL")
                    nc.scalar.activation(SL[:], xs[:], AF.Silu)
                    slv = SL[:].rearrange("p (h w) -> p h w", w=WW)

                    def mk_rhs(off, t, h0, dj, _B=Bviews, _s=slv, _w=wtiles):
                        if t < 2:
                            return (_w[off * 3 + t],
                                    _B[t][:, h0:h0 + 15, dj:dj + WO])
                        return (_w[off * 3 + 2][0:32],
                                _s[:, h0:h0 + 15, dj:dj + WO])

                    banks(im, mk_rhs)
                else:
                    views = []
                    for t in range(3):
                        ft = fp.tile([128, PIX], f32, tag=f"f{t}")
                        src = (xt[:, sl].unsqueeze(0)
                               .broadcast_to((4, 32, PIX)))
                        nc.sync.dma_start(ft[:], src)
                        nsp = 128 if t < 2 else 96
                        nc.scalar.activation(
                            ft[:nsp], ft[:nsp], AF.Relu,
                            bias=ct[:nsp, t:t + 1], scale=1.0 / HGRID)
                        if t == 2:
                            nc.scalar.activation(
                                ft[96:128], ft[96:128], AF.Silu)
                        nc.vector.tensor_scalar_min(
                            ft[:nsp], ft[:nsp], ct[:nsp, 3 + t:4 + t])
                        sq = sp.tile([128, PIX], f32, tag="sq")
                        nc.vector.tensor_mul(sq[:nsp], ft[:nsp], ft[:nsp])
                        nc.vector.tensor_mul(ft[:nsp], sq[:nsp], ft[:nsp])
                        views.append(
                            ft[:].rearrange("p (h w) -> p h w", w=WW))

                    def mk_rhs(off, t, h0, dj, _v=views, _w=wtiles):
                        return (_w[off * 3 + t],
                                _v[t][:, h0:h0 + 15, dj:dj + WO])

                    banks(im, mk_rhs)

    nc.compile()
    return nc


def _prep_fp32(spline_kernel, scale_factor):
    """Truncated-power-folded weights, r-major (r, c) K layout."""
    w = spline_kernel.astype(np.float64) * scale_factor.astype(np.float64)[:, None, :]
    cm = np.array([1.0, -4.0, 6.0, -4.0, 1.0], np.float64) / 6.0
    Wp = np.zeros((KH * KW, NFEAT, C, F), np.float64)
    wr = w.reshape(KH * KW, C, 8, F)
    for r in range(NR):
        for m in range(5):
            k = r - m
            if 0 <= k < 8:
                Wp[:, r] += wr[:, :, k] * cm[m]
    Wp[:, NR] = scale_factor.astype(np.float64).reshape(KH * KW, C, F)
    return Wp.reshape(NMM, 128, F)


def _prep_basis(spline_kernel, scale_factor):
    """Raw spline weights /6, (4k x 32c) K layout + silu chunks."""
    w6 = (spline_kernel.astype(np.float64)
          * scale_factor.astype(np.float64)[:, None, :]) / 6.0
    w6 = w6.reshape(KH * KW, C, 8, F)
    sf = scale_factor.astype(np.float64).reshape(KH * KW, C, F)
    Wt = np.zeros((NMM + 2, 128, F), np.float64)
    for off in range(KH * KW):
        for g in range(2):
            blk = w6[off, :, 4 * g:4 * g + 4]            # (32c, 4k, F)
            Wt[off * 3 + g] = blk.transpose(1, 0, 2).reshape(128, F)
        Wt[off * 3 + 2, 0:32] = sf[off]
    # banded combine matrices: B[p_out] = sum_in M[p_in, p_out] T[p_in]
    cm = np.array([1.0, -4.0, 6.0, -4.0, 1.0])
    pin = np.arange(128)[:, None]
    pout = np.arange(128)[None, :]
    same_c = (pin % 32) == (pout % 32)
    for j, base in ((NMM, 0), (NMM + 1, 4)):
        m = base + pin // 32 - pout // 32
        val = np.where((m >= 0) & (m <= 4) & same_c, cm[np.clip(m, 0, 4)], 0.0)
        Wt[j] = val
    return Wt


def _prep_static(mode, spline_kernel, scale_factor, kan_bias, conv_bias):
    if mode == "basis":
        Wt = _prep_basis(spline_kernel, scale_factor)
    else:
        Wt = _prep_fp32(spline_kernel, scale_factor)
    nch = Wt.shape[0]
    wt = np.ascontiguousarray(
        Wt.transpose(1, 0, 2).reshape(128, nch * F), np.float32)

    consts = np.zeros((128, 8), np.float32)
    p = np.arange(128)
    for t in range(3):
        r = 4 * t + p // 32
        consts[:, t] = -(T0 + HGRID * r) / HGRID           # 5.5 - r
        consts[:, 3 + t] = NR - r                           # 11 - r
    consts[:, 6] = (kan_bias.astype(np.float64)
                    + conv_bias.astype(np.float64)).astype(np.float32)
    return wt, consts


def kernel(x, spline_kernel, scale_factor, kan_bias, conv_bias):
    from concourse import bass_utils

    key = f"nc_{MODE}"
    if key not in _cache:
        _cache[key] = _build_program(MODE)
    nc = _cache[key]

    wt, consts = _prep_static(MODE, spline_kernel, scale_factor,
                              kan_bias, conv_bias)

    in_maps = []
    for c in range(N_CORES):
        xc = x[c * BPC:(c + 1) * BPC]                      # (4,32,32,32)
        xtc = np.ascontiguousarray(
            xc.transpose(3, 0, 1, 2).reshape(C, BPC * PIX), np.float32
        )
        in_maps.append({"xt": xtc, "wt": wt, "consts": consts})

    res = bass_utils.run_bass_kernel_spmd(
        nc, in_maps, core_ids=list(range(N_CORES)),
        **_cache.get("run_kwargs", {})
    )
    _cache["last_result"] = res

    out = np.empty((B, HO, WO, F), np.float32)
    for c in range(N_CORES):
        yc = res.results[c]["y"]                           # (128, 3600)
        out[c * BPC:(c + 1) * BPC] = (
            yc.reshape(F, BPC, HO, WO).transpose(1, 2, 3, 0)
        )
    return out


# revision 25
# speedup vs baseline: 1.0799x; 1.0799x over previous
"""Conv2D-KAN Trainium2 kernel (8-core data-parallel SPMD).

Formulation
-----------
The reference computes, per 3x3 patch (N = B*30*30 patches, in_size = 288):
    out[n,o] = sum_{i,k} sb[n,i,k] * (spline_kernel*scale)[i,k,o]
             + silu(xf) @ scale_factor + biases
where sb is a cubic B-spline basis (8 funcs) over a uniform grid
(knots t_r = -2.2 + 0.4 r, r = 0..11, h = 0.4).

Key identities:
 1. Basis values depend only on the underlying *pixel*, not the patch
    (patch extraction is a gather), so features are computed per pixel
    (8x less elementwise work than per-patch).
 2. Uniform cubic B-splines decompose over truncated powers:
        B_k(x) = (1/6) sum_{m=0..4} cm_m T_{k+m}(x), cm = [1,-4,6,-4,1]
        T_r(x) = min(relu((x - t_r)/h), 11-r)^3
    The clamp at 11-r makes every B_k *exactly* zero outside the grid
    (integer cancellation), matching the reference's out-of-range
    behaviour without masks, and T_11 == 0 so only r = 0..10 exist.
 3. The whole op is then a 3x3 convolution with 128 filters over
    pixel-feature channels, done as accumulating 128-K matmuls into
    PSUM banks of [128 filters, 450 patches].

Two modes:
 * "fp32"  — features are the 11 truncated cubes + silu per channel
             (384 = 3x128 K-chunks per offset, 27 matmuls per bank),
             blending folded into the weights. Full fp32 matmuls
             (4 cyc/row). Max rel err ~1e-5.
 * "basis" — the blending T -> B_k happens on DVE in fp32 (exact), so
             the matmul operands are the well-conditioned basis values
             (<= 4) and the matmuls run in float32r (TF32-like, 1-pass,
             ~1.4 cyc/row). 8 basis + silu -> 2x128 + 32 K-chunks per
             offset, 27 matmuls per bank. Rel err ~ a few 1e-5.

Each core processes 4 images; output [128, 3600] per core is
transposed on host.
"""

import sys

sys.path.insert(0, "/opt/trn_rl_repo")

import numpy as np

N_CORES = 8
B, HH, WW, C = 32, 32, 32, 32
F = 128
KH = KW = 3
HO, WO = HH - KH + 1, WW - KW + 1          # 30, 30
BPC = B // N_CORES                          # images per core = 4
PIX = HH * WW                               # 1024 pixels per image
NPC = BPC * HO * WO                         # 3600 patches per core
NBANK = 2 * BPC                             # 8 psum banks
BANKN = NPC // NBANK                        # 450
HGRID = 0.4
T0 = -2.2                                   # first knot
NR = 11                                     # truncated-cube features
NFEAT = 12                                  # + silu
NMM = 27                                    # matmuls per bank (both modes)

MODE = "fp32"  # "fp32" | "basis"

_cache = {}


def _build_program(mode):
    import concourse.bacc as bacc
    import concourse.mybir as mybir
    import concourse.tile as tile

    f32 = mybir.dt.float32
    f32r = mybir.dt.float32r
    AF = mybir.ActivationFunctionType
    basis = mode == "basis"

    nch = NMM + 2 if basis else NMM
    nc = bacc.Bacc("TRN2", target_bir_lowering=False, debug=False)
    xt = nc.dram_tensor("xt", [C, BPC * PIX], f32, kind="ExternalInput").ap()
    # weights: [128 partitions, nch * F] -> one contiguous DMA
    wt = nc.dram_tensor("wt", [128, nch * F], f32, kind="ExternalInput").ap()
    consts = nc.dram_tensor("consts", [128, 8], f32, kind="ExternalInput").ap()
    y = nc.dram_tensor("y", [F, NPC], f32, kind="ExternalOutput").ap()

    with tile.TileContext(nc) as tc:
        with (
            tc.tile_pool(name="wp", bufs=1) as wp,
            tc.tile_pool(name="cp", bufs=1) as cp,
            tc.tile_pool(name="fp", bufs=3) as fp,
            tc.tile_pool(name="sp", bufs=3) as sp,
            tc.tile_pool(name="op", bufs=1) as op_,
            tc.tile_pool(name="pp", bufs=4, space="PSUM") as pp,
        ):
            ct = cp.tile([128, 8], f32)
            nc.gpsimd.dma_start(ct[:], consts[:])

            # weights on a separate DMA queue so the first feature tiles
            # (sync queue) are not stuck behind the 1.9MB weight load;
            # chunk 0 arrives first in its own small transfer.
            wbig = wp.tile([128, nch * F], f32, tag="wbig")
            nc.gpsimd.dma_start(wbig[:, :F], wt[:, :F])
            nc.gpsimd.dma_start(wbig[:, F:], wt[:, F:])
            if basis:
                wrbig = wp.tile([128, NMM * F], f32r, tag="wrbig")
                nc.vector.tensor_copy(wrbig[:], wbig[:, :NMM * F])
                wtiles = [wrbig[:, i * F:(i + 1) * F] for i in range(NMM)]
                Ma = wbig[:, NMM * F:(NMM + 1) * F]
                Mb = wbig[:, (NMM + 1) * F:(NMM + 2) * F]
            else:
                wtiles = [wbig[:, i * F:(i + 1) * F] for i in range(NMM)]

            out_t = op_.tile([F, NPC], f32)

            def banks(im, mk_rhs):
                for half in range(2):
                    ps = pp.tile([F, BANKN], f32, tag="ps")
                    k = 0
                    for off in range(KH * KW):
                        di, dj = divmod(off, KW)
                        h0 = half * 15 + di
                        for t in range(3):
                            lhsT, rhs = mk_rhs(off, t, h0, dj)
                            nc.tensor.matmul(
                                ps[:], lhsT, rhs,
                                start=(k == 0), stop=(k == NMM - 1),
                            )
                            k += 1
                    s = (im * 2 + half) * BANKN
                    nc.scalar.activation(
                        out_t[:, s:s + BANKN], ps[:], AF.Identity,
                        bias=ct[:, 6:7], scale=1.0,
                    )
                    nc.sync.dma_start(y[:, s:s + BANKN], out_t[:, s:s + BANKN])

            for im in range(BPC):
                sl = slice(im * PIX, (im + 1) * PIX)
                if basis:
                    # --- T tiles (same r-major 4r x 32c layout as fp32 mode)
                    Ts = []
                    for t in range(3):
                        T = fp.tile([128, PIX], f32, tag=f"T{t}")
                        for rep in range(4):
                            nc.sync.dma_start(
                                T[32 * rep:32 * rep + 32], xt[:, sl])
                        nc.scalar.activation(
                            T[:], T[:], AF.Relu,
                            bias=ct[:, t:t + 1], scale=1.0 / HGRID)
                        nc.vector.tensor_scalar_min(
                            T[:], T[:], ct[:, 3 + t:4 + t])
                        sq = sp.tile([128, PIX], f32, tag="sq")
                        nc.scalar.activation(sq[:], T[:], AF.Square)
                        nc.vector.tensor_mul(T[:], sq[:], T[:])
                        Ts.append(T)
                    # --- combine B_k = sum_m cm_m T_{k+m} on PE:
                    # two banded constant matrices contract the r dim
                    # (engines cannot read shifted partition windows).
                    Bviews = []
                    for g in range(2):
                        Bt = fp.tile([128, PIX], f32r, tag=f"B{g}")
                        for hf in range(2):
                            hs = slice(hf * 512, (hf + 1) * 512)
                            bp = pp.tile([128, 512], f32, tag="psB")
                            nc.tensor.matmul(bp[:], Ma, Ts[g][:, hs],
                                             start=True, stop=False)
                            nc.tensor.matmul(bp[:], Mb, Ts[g + 1][:, hs],
                                             start=False, stop=True)
                            nc.scalar.activation(Bt[:, hs], bp[:], AF.Copy)
                        Bviews.append(
                            Bt[:].rearrange("p (h w) -> p h w", w=WW))
                    # --- silu ---
                    xs = sp.tile([32, PIX], f32, tag="xs")
                    nc.sync.dma_start(xs[:], xt[:, sl])
                    SL = fp.tile([32, PIX], f32r, tag="SL")
                    nc.scalar.activation(SL[:], xs[:], AF.Silu)
                    slv = SL[:].rearrange("p (h w) -> p h w", w=WW)

                    def mk_rhs(off, t, h0, dj, _B=Bviews, _s=slv, _w=wtiles):
                        if t < 2:
                            return (_w[off * 3 + t],
                                    _B[t][:, h0:h0 + 15, dj:dj + WO])
                        return (_w[off * 3 + 2][0:32],
                                _s[:, h0:h0 + 15, dj:dj + WO])

                    banks(im, mk_rhs)
                else:
                    views = []
                    for t in range(3):
                        ft = fp.tile([128, PIX], f32, tag=f"f{t}")
                        for rep in range(4):
                            nc.sync.dma_start(
                                ft[32 * rep:32 * rep + 32], xt[:, sl])
                        nsp = 128 if t < 2 else 96
                        nc.scalar.activation(
                            ft[:nsp], ft[:nsp], AF.Relu,
                            bias=ct[:nsp, t:t + 1], scale=1.0 / HGRID)
                        if t == 2:
                            nc.scalar.activation(
                                ft[96:128], ft[96:128], AF.Silu)
                        nc.vector.tensor_scalar_min(
                            ft[:nsp], ft[:nsp], ct[:nsp, 3 + t:4 + t])
                        sq = sp.tile([128, PIX], f32, tag="sq")
                        nc.vector.tensor_mul(sq[:nsp], ft[:nsp], ft[:nsp])
                        nc.vector.tensor_mul(ft[:nsp], sq[:nsp], ft[:nsp])
                        views.append(
                            ft[:].rearrange("p (h w) -> p h w", w=WW))

                    def mk_rhs(off, t, h0, dj, _v=views, _w=wtiles):
                        return (_w[off * 3 + t],
                                _v[t][:, h0:h0 + 15, dj:dj + WO])

                    banks(im, mk_rhs)

    nc.compile()
    return nc


def _prep_fp32(spline_kernel, scale_factor):
    """Truncated-power-folded weights, r-major (r, c) K layout."""
    w = spline_kernel.astype(np.float64) * scale_factor.astype(np.float64)[:, None, :]
    cm = np.array([1.0, -4.0, 6.0, -4.0, 1.0], np.float64) / 6.0
    Wp = np.zeros((KH * KW, NFEAT, C, F), np.float64)
    wr = w.reshape(KH * KW, C, 8, F)
    for r in range(NR):
        for m in range(5):
            k = r - m
            if 0 <= k < 8:
                Wp[:, r] += wr[:, :, k] * cm[m]
    Wp[:, NR] = scale_factor.astype(np.float64).reshape(KH * KW, C, F)
    return Wp.reshape(NMM, 128, F)


def _prep_basis(spline_kernel, scale_factor):
    """Raw spline weights /6, (4k x 32c) K layout + silu chunks."""
    w6 = (spline_kernel.astype(np.float64)
          * scale_factor.astype(np.float64)[:, None, :]) / 6.0
    w6 = w6.reshape(KH * KW, C, 8, F)
    sf = scale_factor.astype(np.float64).reshape(KH * KW, C, F)
    Wt = np.zeros((NMM + 2, 128, F), np.float64)
    for off in range(KH * KW):
        for g in range(2):
            blk = w6[off, :, 4 * g:4 * g + 4]            # (32c, 4k, F)
            Wt[off * 3 + g] = blk.transpose(1, 0, 2).reshape(128, F)
        Wt[off * 3 + 2, 0:32] = sf[off]
    # banded combine matrices: B[p_out] = sum_in M[p_in, p_out] T[p_in]
    cm = np.array([1.0, -4.0, 6.0, -4.0, 1.0])
    pin = np.arange(128)[:, None]
    pout = np.arange(128)[None, :]
    same_c = (pin % 32) == (pout % 32)
    for j, base in ((NMM, 0), (NMM + 1, 4)):
        m = base + pin // 32 - pout // 32
        val = np.where((m >= 0) & (m <= 4) & same_c, cm[np.clip(m, 0, 4)], 0.0)
        Wt[j] = val
    return Wt


def _prep_static(mode, spline_kernel, scale_factor, kan_bias, conv_bias):
    if mode == "basis":
        Wt = _prep_basis(spline_kernel, scale_factor)
    else:
        Wt = _prep_fp32(spline_kernel, scale_factor)
    nch = Wt.shape[0]
    wt = np.ascontiguousarray(
        Wt.transpose(1, 0, 2).reshape(128, nch * F), np.float32)

    consts = np.zeros((128, 8), np.float32)
    p = np.arange(128)
    for t in range(3):
        r = 4 * t + p // 32
        consts[:, t] = -(T0 + HGRID * r) / HGRID           # 5.5 - r
        consts[:, 3 + t] = NR - r                           # 11 - r
    consts[:, 6] = (kan_bias.astype(np.float64)
                    + conv_bias.astype(np.float64)).astype(np.float32)
    return wt, consts


def kernel(x, spline_kernel, scale_factor, kan_bias, conv_bias):
    from concourse import bass_utils

    key = f"nc_{MODE}"
    if key not in _cache:
        _cache[key] = _build_program(MODE)
    nc = _cache[key]

    wt, consts = _prep_static(MODE, spline_kernel, scale_factor,
                              kan_bias, conv_bias)

    in_maps = []
    for c in range(N_CORES):
        xc = x[c * BPC:(c + 1) * BPC]                      # (4,32,32,32)
        xtc = np.ascontiguousarray(
            xc.transpose(3, 0, 1, 2).reshape(C, BPC * PIX), np.float32
        )
        in_maps.append({"xt": xtc, "wt": wt, "consts": consts})

    res = bass_utils.run_bass_kernel_spmd(
        nc, in_maps, core_ids=list(range(N_CORES)),
        **_cache.get("run_kwargs", {})
    )
    _cache["last_result"] = res

    out = np.empty((B, HO, WO, F), np.float32)
    for c in range(N_CORES):
        yc = res.results[c]["y"]                           # (128, 3600)
        out[c * BPC:(c + 1) * BPC] = (
            yc.reshape(F, BPC, HO, WO).transpose(1, 2, 3, 0)
        )
    return out


# revision 26
# speedup vs baseline: 1.0832x; 1.0030x over previous
"""Conv2D-KAN Trainium2 kernel (8-core data-parallel SPMD).

Formulation
-----------
The reference computes, per 3x3 patch (N = B*30*30 patches, in_size = 288):
    out[n,o] = sum_{i,k} sb[n,i,k] * (spline_kernel*scale)[i,k,o]
             + silu(xf) @ scale_factor + biases
where sb is a cubic B-spline basis (8 funcs) over a uniform grid
(knots t_r = -2.2 + 0.4 r, r = 0..11, h = 0.4).

Key identities:
 1. Basis values depend only on the underlying *pixel*, not the patch
    (patch extraction is a gather), so features are computed per pixel
    (8x less elementwise work than per-patch).
 2. Uniform cubic B-splines decompose over truncated powers:
        B_k(x) = (1/6) sum_{m=0..4} cm_m T_{k+m}(x), cm = [1,-4,6,-4,1]
        T_r(x) = min(relu((x - t_r)/h), 11-r)^3
    The clamp at 11-r makes every B_k *exactly* zero outside the grid
    (integer cancellation), matching the reference's out-of-range
    behaviour without masks, and T_11 == 0 so only r = 0..10 exist.
 3. The whole op is then a 3x3 convolution with 128 filters over
    pixel-feature channels, done as accumulating 128-K matmuls into
    PSUM banks of [128 filters, 450 patches].

Two modes:
 * "fp32"  — features are the 11 truncated cubes + silu per channel
             (384 = 3x128 K-chunks per offset, 27 matmuls per bank),
             blending folded into the weights. Full fp32 matmuls
             (4 cyc/row). Max rel err ~1e-5.
 * "basis" — the blending T -> B_k happens on DVE in fp32 (exact), so
             the matmul operands are the well-conditioned basis values
             (<= 4) and the matmuls run in float32r (TF32-like, 1-pass,
             ~1.4 cyc/row). 8 basis + silu -> 2x128 + 32 K-chunks per
             offset, 27 matmuls per bank. Rel err ~ a few 1e-5.

Each core processes 4 images; output [128, 3600] per core is
transposed on host.
"""

import sys

sys.path.insert(0, "/opt/trn_rl_repo")

import numpy as np

N_CORES = 8
B, HH, WW, C = 32, 32, 32, 32
F = 128
KH = KW = 3
HO, WO = HH - KH + 1, WW - KW + 1          # 30, 30
BPC = B // N_CORES                          # images per core = 4
PIX = HH * WW                               # 1024 pixels per image
NPC = BPC * HO * WO                         # 3600 patches per core
NBANK = 2 * BPC                             # 8 psum banks
BANKN = NPC // NBANK                        # 450
HGRID = 0.4
T0 = -2.2                                   # first knot
NR = 11                                     # truncated-cube features
NFEAT = 12                                  # + silu
NMM = 27                                    # matmuls per bank (both modes)

MODE = "fp32"  # "fp32" | "basis"

_cache = {}


def _build_program(mode):
    import concourse.bacc as bacc
    import concourse.mybir as mybir
    import concourse.tile as tile

    f32 = mybir.dt.float32
    f32r = mybir.dt.float32r
    AF = mybir.ActivationFunctionType
    basis = mode == "basis"

    nch = NMM + 2 if basis else NMM
    nc = bacc.Bacc("TRN2", target_bir_lowering=False, debug=False)
    xt = nc.dram_tensor("xt", [C, BPC * PIX], f32, kind="ExternalInput").ap()
    # weights: [128 partitions, nch * F] -> one contiguous DMA
    wt = nc.dram_tensor("wt", [128, nch * F], f32, kind="ExternalInput").ap()
    consts = nc.dram_tensor("consts", [128, 8], f32, kind="ExternalInput").ap()
    y = nc.dram_tensor("y", [F, NPC], f32, kind="ExternalOutput").ap()

    with tile.TileContext(nc) as tc:
        with (
            tc.tile_pool(name="wp", bufs=1) as wp,
            tc.tile_pool(name="cp", bufs=1) as cp,
            tc.tile_pool(name="fp", bufs=3) as fp,
            tc.tile_pool(name="sp", bufs=3) as sp,
            tc.tile_pool(name="op", bufs=1) as op_,
            tc.tile_pool(name="pp", bufs=4, space="PSUM") as pp,
        ):
            ct = cp.tile([128, 8], f32)
            nc.gpsimd.dma_start(ct[:], consts[:])

            # weights on a separate DMA queue so the first feature tiles
            # (sync queue) are not stuck behind the 1.9MB weight load;
            # chunk 0 arrives first in its own small transfer.
            wbig = wp.tile([128, nch * F], f32, tag="wbig")
            nc.gpsimd.dma_start(wbig[:, :F], wt[:, :F])
            nc.gpsimd.dma_start(wbig[:, F:], wt[:, F:])

            # warm up the ACT table set (silu's set also carries relu /
            # copy / identity / square fillers) so the ~1.3us table load
            # happens before the first feature tile is ready.
            warm = cp.tile([1, 1], f32, tag="warm")
            nc.scalar.activation(warm[:], ct[:1, :1], AF.Silu)
            if basis:
                wrbig = wp.tile([128, NMM * F], f32r, tag="wrbig")
                nc.vector.tensor_copy(wrbig[:], wbig[:, :NMM * F])
                wtiles = [wrbig[:, i * F:(i + 1) * F] for i in range(NMM)]
                Ma = wbig[:, NMM * F:(NMM + 1) * F]
                Mb = wbig[:, (NMM + 1) * F:(NMM + 2) * F]
            else:
                wtiles = [wbig[:, i * F:(i + 1) * F] for i in range(NMM)]

            out_t = op_.tile([F, NPC], f32)

            def banks(im, mk_rhs):
                for half in range(2):
                    ps = pp.tile([F, BANKN], f32, tag="ps")
                    k = 0
                    for off in range(KH * KW):
                        di, dj = divmod(off, KW)
                        h0 = half * 15 + di
                        for t in range(3):
                            lhsT, rhs = mk_rhs(off, t, h0, dj)
                            nc.tensor.matmul(
                                ps[:], lhsT, rhs,
                                start=(k == 0), stop=(k == NMM - 1),
                            )
                            k += 1
                    s = (im * 2 + half) * BANKN
                    nc.scalar.activation(
                        out_t[:, s:s + BANKN], ps[:], AF.Identity,
                        bias=ct[:, 6:7], scale=1.0,
                    )
                    nc.sync.dma_start(y[:, s:s + BANKN], out_t[:, s:s + BANKN])

            for im in range(BPC):
                sl = slice(im * PIX, (im + 1) * PIX)
                if basis:
                    # --- T tiles (same r-major 4r x 32c layout as fp32 mode)
                    Ts = []
                    for t in range(3):
                        T = fp.tile([128, PIX], f32, tag=f"T{t}")
                        for rep in range(4):
                            nc.sync.dma_start(
                                T[32 * rep:32 * rep + 32], xt[:, sl])
                        nc.scalar.activation(
                            T[:], T[:], AF.Relu,
                            bias=ct[:, t:t + 1], scale=1.0 / HGRID)
                        nc.vector.tensor_scalar_min(
                            T[:], T[:], ct[:, 3 + t:4 + t])
                        sq = sp.tile([128, PIX], f32, tag="sq")
                        nc.scalar.activation(sq[:], T[:], AF.Square)
                        nc.vector.tensor_mul(T[:], sq[:], T[:])
                        Ts.append(T)
                    # --- combine B_k = sum_m cm_m T_{k+m} on PE:
                    # two banded constant matrices contract the r dim
                    # (engines cannot read shifted partition windows).
                    Bviews = []
                    for g in range(2):
                        Bt = fp.tile([128, PIX], f32r, tag=f"B{g}")
                        for hf in range(2):
                            hs = slice(hf * 512, (hf + 1) * 512)
                            bp = pp.tile([128, 512], f32, tag="psB")
                            nc.tensor.matmul(bp[:], Ma, Ts[g][:, hs],
                                             start=True, stop=False)
                            nc.tensor.matmul(bp[:], Mb, Ts[g + 1][:, hs],
                                             start=False, stop=True)
                            nc.scalar.activation(Bt[:, hs], bp[:], AF.Copy)
                        Bviews.append(
                            Bt[:].rearrange("p (h w) -> p h w", w=WW))
                    # --- silu ---
                    xs = sp.tile([32, PIX], f32, tag="xs")
                    nc.sync.dma_start(xs[:], xt[:, sl])
                    SL = fp.tile([32, PIX], f32r, tag="SL")
                    nc.scalar.activation(SL[:], xs[:], AF.Silu)
                    slv = SL[:].rearrange("p (h w) -> p h w", w=WW)

                    def mk_rhs(off, t, h0, dj, _B=Bviews, _s=slv, _w=wtiles):
                        if t < 2:
                            return (_w[off * 3 + t],
                                    _B[t][:, h0:h0 + 15, dj:dj + WO])
                        return (_w[off * 3 + 2][0:32],
                                _s[:, h0:h0 + 15, dj:dj + WO])

                    banks(im, mk_rhs)
                else:
                    views = []
                    for t in range(3):
                        ft = fp.tile([128, PIX], f32, tag=f"f{t}")
                        for rep in range(4):
                            nc.sync.dma_start(
                                ft[32 * rep:32 * rep + 32], xt[:, sl])
                        nsp = 128 if t < 2 else 96
                        nc.scalar.activation(
                            ft[:nsp], ft[:nsp], AF.Relu,
                            bias=ct[:nsp, t:t + 1], scale=1.0 / HGRID)
                        if t == 2:
                            nc.scalar.activation(
                                ft[96:128], ft[96:128], AF.Silu)
                        nc.vector.tensor_scalar_min(
                            ft[:nsp], ft[:nsp], ct[:nsp, 3 + t:4 + t])
                        sq = sp.tile([128, PIX], f32, tag="sq")
                        nc.vector.tensor_mul(sq[:nsp], ft[:nsp], ft[:nsp])
                        nc.vector.tensor_mul(ft[:nsp], sq[:nsp], ft[:nsp])
                        views.append(
                            ft[:].rearrange("p (h w) -> p h w", w=WW))

                    def mk_rhs(off, t, h0, dj, _v=views, _w=wtiles):
                        return (_w[off * 3 + t],
                                _v[t][:, h0:h0 + 15, dj:dj + WO])

                    banks(im, mk_rhs)

    nc.compile()
    return nc


def _prep_fp32(spline_kernel, scale_factor):
    """Truncated-power-folded weights, r-major (r, c) K layout."""
    w = spline_kernel.astype(np.float64) * scale_factor.astype(np.float64)[:, None, :]
    cm = np.array([1.0, -4.0, 6.0, -4.0, 1.0], np.float64) / 6.0
    Wp = np.zeros((KH * KW, NFEAT, C, F), np.float64)
    wr = w.reshape(KH * KW, C, 8, F)
    for r in range(NR):
        for m in range(5):
            k = r - m
            if 0 <= k < 8:
                Wp[:, r] += wr[:, :, k] * cm[m]
    Wp[:, NR] = scale_factor.astype(np.float64).reshape(KH * KW, C, F)
    return Wp.reshape(NMM, 128, F)


def _prep_basis(spline_kernel, scale_factor):
    """Raw spline weights /6, (4k x 32c) K layout + silu chunks."""
    w6 = (spline_kernel.astype(np.float64)
          * scale_factor.astype(np.float64)[:, None, :]) / 6.0
    w6 = w6.reshape(KH * KW, C, 8, F)
    sf = scale_factor.astype(np.float64).reshape(KH * KW, C, F)
    Wt = np.zeros((NMM + 2, 128, F), np.float64)
    for off in range(KH * KW):
        for g in range(2):
            blk = w6[off, :, 4 * g:4 * g + 4]            # (32c, 4k, F)
            Wt[off * 3 + g] = blk.transpose(1, 0, 2).reshape(128, F)
        Wt[off * 3 + 2, 0:32] = sf[off]
    # banded combine matrices: B[p_out] = sum_in M[p_in, p_out] T[p_in]
    cm = np.array([1.0, -4.0, 6.0, -4.0, 1.0])
    pin = np.arange(128)[:, None]
    pout = np.arange(128)[None, :]
    same_c = (pin % 32) == (pout % 32)
    for j, base in ((NMM, 0), (NMM + 1, 4)):
        m = base + pin // 32 - pout // 32
        val = np.where((m >= 0) & (m <= 4) & same_c, cm[np.clip(m, 0, 4)], 0.0)
        Wt[j] = val
    return Wt


def _prep_static(mode, spline_kernel, scale_factor, kan_bias, conv_bias):
    if mode == "basis":
        Wt = _prep_basis(spline_kernel, scale_factor)
    else:
        Wt = _prep_fp32(spline_kernel, scale_factor)
    nch = Wt.shape[0]
    wt = np.ascontiguousarray(
        Wt.transpose(1, 0, 2).reshape(128, nch * F), np.float32)

    consts = np.zeros((128, 8), np.float32)
    p = np.arange(128)
    for t in range(3):
        r = 4 * t + p // 32
        consts[:, t] = -(T0 + HGRID * r) / HGRID           # 5.5 - r
        consts[:, 3 + t] = NR - r                           # 11 - r
    consts[:, 6] = (kan_bias.astype(np.float64)
                    + conv_bias.astype(np.float64)).astype(np.float32)
    return wt, consts


def kernel(x, spline_kernel, scale_factor, kan_bias, conv_bias):
    from concourse import bass_utils

    key = f"nc_{MODE}"
    if key not in _cache:
        _cache[key] = _build_program(MODE)
    nc = _cache[key]

    wt, consts = _prep_static(MODE, spline_kernel, scale_factor,
                              kan_bias, conv_bias)

    in_maps = []
    for c in range(N_CORES):
        xc = x[c * BPC:(c + 1) * BPC]                      # (4,32,32,32)
        xtc = np.ascontiguousarray(
            xc.transpose(3, 0, 1, 2).reshape(C, BPC * PIX), np.float32
        )
        in_maps.append({"xt": xtc, "wt": wt, "consts": consts})

    res = bass_utils.run_bass_kernel_spmd(
        nc, in_maps, core_ids=list(range(N_CORES)),
        **_cache.get("run_kwargs", {})
    )
    _cache["last_result"] = res

    out = np.empty((B, HO, WO, F), np.float32)
    for c in range(N_CORES):
        yc = res.results[c]["y"]                           # (128, 3600)
        out[c * BPC:(c + 1) * BPC] = (
            yc.reshape(F, BPC, HO, WO).transpose(1, 2, 3, 0)
        )
    return out
